# revision 2
# baseline (speedup 1.0000x reference)
"""FAGCN layer (gnn_message_passing) as a Bass/Tile kernel on 8 trn2 NeuronCores.

Strategy: destination-sharded edge parallelism (no collectives).
  - Node dest range is split into 8 contiguous slices of 5120 nodes (40000
    padded to 40960); core k owns all edges whose dest (col) lands in its slice.
  - x is replicated; each core gathers the source rows x[row] it needs with
    dma_gather (int16 indices => per-block lo/hi split at 32768).
  - Edges are bucketed by dest block of 128 nodes; a chunk of 128 edges is
    scattered into the block via a one-hot matmul accumulated in PSUM:
        psum[c, f] += sum_e onehot[e, c] * x_i[e, f],  onehot[e, col_e] = alpha_e
  - alpha_e = tanh(s_e + t_col + b) with s = x_i . w1 computed by a fused
    mul+reduce on DVE, and t = x_loc . w2 + b precomputed per core; the
    per-edge t selection is fused into the ACT engine:
        alphaM[e, c] = tanh(t_bcast[c] + s_e)  (bias = per-partition s)
    and the one-hot build picks column col_e:
        O[e, c] = (iota[c] == col_e) * alphaM[e, c]
  - out = eps * x + (1 - eps) * psum, written per block; host concatenates the
    8 core slices.
"""
import numpy as np
from contextlib import ExitStack

import concourse.bass as bass
import concourse.bacc as bacc
import concourse.mybir as mybir
import concourse.tile as tile
import concourse.bass_utils as bu

P = 128
D = 128
N_NODES = 40000
N_EDGES = 640000
N_CORES = 8
NPAD = 40960
NLOC = NPAD // N_CORES          # 5120 dest nodes per core
NBLK = NLOC // P                # 40 dest blocks per core
SPLIT = 32768                   # int16 gather index limit
GRP = 4                         # dest blocks per gather group
NGRP = NBLK // GRP


def _cdiv(a, b):
    return (a + b - 1) // b


def _wrap16(idx):
    """[n] int -> [128, n/16] int16 in the dma_gather wrapped layout."""
    n = len(idx)
    assert n % 16 == 0
    w = idx.reshape(n // 16, 16).T.astype(np.int16)
    return np.ascontiguousarray(np.tile(w, (8, 1)))


def _host_pack(x, edge_index):
    """Bucket edges by (dest block, lo/hi source half); build per-core inputs."""
    row = np.asarray(edge_index[0], dtype=np.int64)
    col = np.asarray(edge_index[1], dtype=np.int64)

    bg = col >> 7                       # global dest block, 0..312
    hi = (row >= SPLIT).astype(np.int64)
    key = bg * 2 + hi
    order = np.argsort(key, kind="stable")
    row_s, col_s, key_s = row[order], col[order], key[order]
    counts = np.bincount(key_s, minlength=NBLK * N_CORES * 2)
    starts = np.concatenate([[0], np.cumsum(counts)])

    n_lo = counts[0::2].reshape(N_CORES, NBLK)   # [core, blk]
    n_hi = counts[1::2].reshape(N_CORES, NBLK)
    C_lo = np.maximum(1, _cdiv(n_lo.max(axis=0), P))   # [blk]
    C_hi = _cdiv(n_hi.max(axis=0), P)                   # [blk], may be 0

    # group geometry (same for all cores)
    meta = {"C_lo": C_lo, "C_hi": C_hi, "groups": []}
    for g in range(NGRP):
        blks = list(range(g * GRP, (g + 1) * GRP))
        K_lo = int(C_lo[blks].sum())
        K_hi = int(C_hi[blks].sum())
        # chunk id -> (block, kind) map in slot order: lo region then hi region
        chunk_blk = []
        for b in blks:
            chunk_blk += [b] * int(C_lo[b])
        for b in blks:
            chunk_blk += [b] * int(C_hi[b])
        meta["groups"].append({"blks": blks, "K_lo": K_lo, "K_hi": K_hi,
                               "K": K_lo + K_hi, "chunk_blk": chunk_blk})

    in_maps = []
    for core in range(N_CORES):
        m = {}
        for g in range(NGRP):
            gi = meta["groups"][g]
            K_lo, K_hi, K = gi["K_lo"], gi["K_hi"], gi["K"]
            idx_lo = np.zeros(K_lo * P, dtype=np.int64)
            idx_hi = np.zeros(K_hi * P, dtype=np.int64)
            colf = np.full(K * P, 999.0, dtype=np.float32)  # slot-ordered
            off_lo = 0
            off_hi = 0
            for kind in (0, 1):
                for b in gi["blks"]:
                    bg_idx = (core * NBLK + b)
                    if bg_idx >= 313:
                        n = 0
                    else:
                        k2 = bg_idx * 2 + kind
                        n = counts[k2]
                    if kind == 0:
                        cap = int(C_lo[b]) * P
                        if n:
                            s0 = starts[k2]
                            idx_lo[off_lo:off_lo + n] = row_s[s0:s0 + n]
                            colf[off_lo:off_lo + n] = (col_s[s0:s0 + n] & 127)
                        off_lo += cap
                    else:
                        cap = int(C_hi[b]) * P
                        if n:
                            s0 = starts[k2]
                            idx_hi[off_hi:off_hi + n] = row_s[s0:s0 + n] - SPLIT
                            colf[K_lo * P + off_hi:K_lo * P + off_hi + n] = (col_s[s0:s0 + n] & 127)
                        off_hi += cap
            # colf covers lo region [0, K_lo*P) then hi region
            m[f"idxlo{g}"] = _wrap16(idx_lo)
            if K_hi:
                m[f"idxhi{g}"] = _wrap16(idx_hi)
            m[f"colf{g}"] = np.ascontiguousarray(
                colf.reshape(K, P).T).astype(np.float32)        # [128, K]
        in_maps.append(m)

    # sanity: each (core,block,kind) run fits its cap
    assert (n_lo <= C_lo[None, :] * P).all() and (n_hi <= C_hi[None, :] * P).all()
    return in_maps, meta


def _build_program(meta, att_b, eps_v):
    nc = bacc.Bacc("TRN2", target_bir_lowering=False, debug=False,
                   enable_asserts=False)

    x_d = nc.dram_tensor("x", (N_NODES, D), mybir.dt.float32, kind="ExternalInput")
    xloc_d = nc.dram_tensor("xloc", (NLOC, D), mybir.dt.float32, kind="ExternalInput")
    w1_d = nc.dram_tensor("w1", (P, D), mybir.dt.float32, kind="ExternalInput")
    w2_d = nc.dram_tensor("w2", (P, D), mybir.dt.float32, kind="ExternalInput")
    iota_d = nc.dram_tensor("iota", (P, D), mybir.dt.float32, kind="ExternalInput")
    grp_in = []
    for g, gi in enumerate(meta["groups"]):
        K_lo, K_hi, K = gi["K_lo"], gi["K_hi"], gi["K"]
        dlo = nc.dram_tensor(f"idxlo{g}", (P, K_lo * 8), mybir.dt.int16, kind="ExternalInput")
        dhi = (nc.dram_tensor(f"idxhi{g}", (P, K_hi * 8), mybir.dt.int16, kind="ExternalInput")
               if K_hi else None)
        dcol = nc.dram_tensor(f"colf{g}", (P, K), mybir.dt.float32, kind="ExternalInput")
        grp_in.append((dlo, dhi, dcol))
    out_d = nc.dram_tensor("out", (NLOC, D), mybir.dt.float32, kind="ExternalOutput")

    K_max = max(gi["K"] for gi in meta["groups"])

    with tile.TileContext(nc) as tc, ExitStack() as ctx:
        cpool = ctx.enter_context(tc.tile_pool(name="consts", bufs=1))
        spool = ctx.enter_context(tc.tile_pool(name="small", bufs=2))
        gpool = ctx.enter_context(tc.tile_pool(name="gather", bufs=2))
        mpool = ctx.enter_context(tc.tile_pool(name="mul", bufs=1))
        opool = ctx.enter_context(tc.tile_pool(name="oh", bufs=4))
        apool = ctx.enter_context(tc.tile_pool(name="alpha", bufs=4))
        ppool = ctx.enter_context(tc.tile_pool(name="psum", bufs=4, space="PSUM"))
        dpool = ctx.enter_context(tc.tile_pool(name="dram", bufs=1, space="DRAM"))

        w1_sb = cpool.tile([P, D], mybir.dt.float32)
        w2_sb = cpool.tile([P, D], mybir.dt.float32)
        iota_sb = cpool.tile([P, D], mybir.dt.float32)
        nc.sync.dma_start(out=w1_sb[:], in_=w1_d.ap())
        nc.sync.dma_start(out=w2_sb[:], in_=w2_d.ap())
        nc.sync.dma_start(out=iota_sb[:], in_=iota_d.ap())

        # ---- prologue: t = x_loc @ w2 + b  -> t_dram [NLOC, 1] ----
        t_dram = dpool.tile([NLOC, 1], mybir.dt.float32)
        t_all = cpool.tile([P, NBLK], mybir.dt.float32)
        for b in range(NBLK):
            xb = spool.tile([P, D], mybir.dt.float32, tag="xb_pro")
            nc.sync.dma_start(out=xb[:], in_=xloc_d.ap()[b * P:(b + 1) * P, :])
            mulb = spool.tile([P, D], mybir.dt.float32, tag="mulb_pro")
            nc.vector.tensor_tensor(out=mulb[:], in0=xb[:], in1=w2_sb[:],
                                    op=mybir.AluOpType.mult)
            nc.vector.tensor_reduce(out=t_all[:, b:b + 1], in_=mulb[:],
                                    axis=mybir.AxisListType.X, op=mybir.AluOpType.add)
        nc.vector.tensor_scalar_add(out=t_all[:], in0=t_all[:], scalar1=float(att_b))
        nc.sync.dma_start(out=t_dram[:].rearrange("(b p) one -> p (b one)", p=P),
                          in_=t_all[:])

        # ---- main: per group gather + per block one-hot matmul ----
        for g, gi in enumerate(meta["groups"]):
            K_lo, K_hi, K = gi["K_lo"], gi["K_hi"], gi["K"]
            dlo, dhi, dcol = grp_in[g]

            xi = gpool.tile([P, K_max, D], mybir.dt.float32, tag="xi")
            idxlo_sb = spool.tile([P, K_lo * 8], mybir.dt.int16, tag="idxlo")
            nc.sync.dma_start(out=idxlo_sb[:], in_=dlo.ap())
            colf_sb = spool.tile([P, K_max], mybir.dt.float32, tag="colf")
            nc.sync.dma_start(out=colf_sb[:, :K], in_=dcol.ap())

            nc.gpsimd.dma_gather(
                out_ap=xi[:, 0:K_lo, :], in_ap=x_d.ap()[0:SPLIT, :],
                idxs_ap=idxlo_sb[:], num_idxs=K_lo * P, num_idxs_reg=K_lo * P,
                elem_size=D, single_packet=False,
            )
            if K_hi:
                idxhi_sb = spool.tile([P, K_hi * 8], mybir.dt.int16, tag="idxhi")
                nc.sync.dma_start(out=idxhi_sb[:], in_=dhi.ap())
                nc.gpsimd.dma_gather(
                    out_ap=xi[:, K_lo:K, :], in_ap=x_d.ap()[SPLIT:N_NODES, :],
                    idxs_ap=idxhi_sb[:], num_idxs=K_hi * P, num_idxs_reg=K_hi * P,
                    elem_size=D, single_packet=False,
                )

            # s = rowwise dot(x_i, w1) for the whole group
            mul3 = mpool.tile([P, K_max, D], mybir.dt.float32, tag="mul3")
            nc.vector.tensor_tensor(
                out=mul3[:, :K, :], in0=xi[:, :K, :],
                in1=w1_sb[:].unsqueeze(1).to_broadcast([P, K, D]),
                op=mybir.AluOpType.mult)
            s_sel = spool.tile([P, K_max], mybir.dt.float32, tag="ssel")
            nc.vector.tensor_reduce(out=s_sel[:, :K], in_=mul3[:, :K, :],
                                    axis=mybir.AxisListType.X, op=mybir.AluOpType.add)

            # chunk ids per block (lo region then hi region)
            for b in gi["blks"]:
                chunks = [k for k, cb in enumerate(gi["chunk_blk"]) if cb == b]
                t_bc = apool.tile([P, P], mybir.dt.float32, tag="tbc")
                nc.sync.dma_start(
                    out=t_bc[:],
                    in_=t_dram[:][b * P:(b + 1) * P, :]
                        .rearrange("p one -> (one p)").unsqueeze(0).to_broadcast([P, P]))
                psum = ppool.tile([P, D], mybir.dt.float32, space="PSUM", tag="ps")
                for j, k in enumerate(chunks):
                    alphaM = apool.tile([P, P], mybir.dt.float32, tag="alphaM")
                    nc.scalar.activation(out=alphaM[:], in_=t_bc[:],
                                         func=mybir.ActivationFunctionType.Tanh,
                                         bias=s_sel[:, k:k + 1], scale=1.0)
                    O = opool.tile([P, P], mybir.dt.float32, tag="O")
                    nc.vector.scalar_tensor_tensor(
                        out=O[:], in0=iota_sb[:], scalar=colf_sb[:, k:k + 1],
                        in1=alphaM[:], op0=mybir.AluOpType.is_equal,
                        op1=mybir.AluOpType.mult)
                    nc.tensor.matmul(out=psum[:], lhsT=O[:], rhs=xi[:, k, :],
                                     start=(j == 0), stop=(j == len(chunks) - 1))
                # out = eps * x_loc + (1 - eps) * psum
                xb = spool.tile([P, D], mybir.dt.float32, tag="xb_mix")
                nc.sync.dma_start(out=xb[:], in_=xloc_d.ap()[b * P:(b + 1) * P, :])
                xbe = spool.tile([P, D], mybir.dt.float32, tag="xbe")
                nc.vector.tensor_scalar_mul(out=xbe[:], in0=xb[:], scalar1=float(eps_v))
                osb = spool.tile([P, D], mybir.dt.float32, tag="osb")
                nc.vector.scalar_tensor_tensor(
                    out=osb[:], in0=psum[:], scalar=float(1.0 - eps_v), in1=xbe[:],
                    op0=mybir.AluOpType.mult, op1=mybir.AluOpType.add)
                nc.sync.dma_start(out=out_d.ap()[b * P:(b + 1) * P, :], in_=osb[:])

    nc.compile()
    return nc


def _run(inputs, trace=False, trace_kwargs=None):
    x = np.ascontiguousarray(np.asarray(inputs["x"], dtype=np.float32))
    edge_index = np.asarray(inputs["edge_index"])
    att_w = np.asarray(inputs["att_w"], dtype=np.float32)
    att_b = float(np.asarray(inputs["att_b"], dtype=np.float32)[0])
    eps_v = float(np.asarray(inputs["eps"], dtype=np.float32)[0])

    in_maps, meta = _host_pack(x, edge_index)

    w1 = np.ascontiguousarray(np.tile(att_w[:, :D], (P, 1)))
    w2 = np.ascontiguousarray(np.tile(att_w[:, D:], (P, 1)))
    iota = np.ascontiguousarray(np.tile(np.arange(P, dtype=np.float32)[None, :], (P, 1)))
    x_pad = np.zeros((NPAD, D), dtype=np.float32)
    x_pad[:N_NODES] = x

    for core in range(N_CORES):
        m = in_maps[core]
        m["x"] = x
        m["xloc"] = np.ascontiguousarray(x_pad[core * NLOC:(core + 1) * NLOC])
        m["w1"] = w1
        m["w2"] = w2
        m["iota"] = iota

    nc = _build_program(meta, att_b, eps_v)
    res = bu.run_bass_kernel_spmd(nc, in_maps, list(range(N_CORES)),
                                  trace=trace, **(trace_kwargs or {}))
    out = np.concatenate([res.results[c]["out"] for c in range(N_CORES)], axis=0)
    return np.ascontiguousarray(out[:N_NODES]), res


def kernel(**inputs) -> np.ndarray:
    out, _ = _run(inputs, trace=False)
    return out


# revision 6
# speedup vs baseline: 1.2397x; 1.2397x over previous
"""FAGCN layer (gnn_message_passing) as a Bass/Tile kernel on 8 trn2 NeuronCores.

Strategy: destination-sharded edge parallelism (no collectives).
  - Node dest range is split into 8 contiguous slices of 5120 nodes (40000
    padded to 40960); core k owns all edges whose dest (col) lands in its slice.
  - x is replicated; each core gathers the source rows x[row] it needs with
    dma_gather (int16 indices => per-block lo/hi split at 32768).
  - Edges are bucketed by dest block of 128 nodes; a chunk of 128 edges is
    scattered into the block via a one-hot matmul accumulated in PSUM:
        psum[c, f] += sum_e onehot[e, c] * x_i[e, f],  onehot[e, col_e] = alpha_e
  - alpha_e = tanh(s_e + t_col + b) with s = x_i . w1 computed by a fused
    mul+reduce on DVE, and t = x_loc . w2 + b precomputed per core; the
    per-edge t selection is fused into the ACT engine:
        alphaM[e, c] = tanh(t_bcast[c] + s_e)  (bias = per-partition s)
    and the one-hot build picks column col_e:
        O[e, c] = (iota[c] == col_e) * alphaM[e, c]
  - out = eps * x + (1 - eps) * psum, written per block; host concatenates the
    8 core slices.
"""
import numpy as np
from contextlib import ExitStack

import concourse.bass as bass
import concourse.bacc as bacc
import concourse.mybir as mybir
import concourse.tile as tile
import concourse.bass_utils as bu

P = 128
D = 128
N_NODES = 40000
N_EDGES = 640000
N_CORES = 8
NPAD = 40960
NLOC = NPAD // N_CORES          # 5120 dest nodes per core
NBLK = NLOC // P                # 40 dest blocks per core
SPLIT = 32768                   # int16 gather index limit
GRP = 4                         # dest blocks per gather group
NGRP = NBLK // GRP


def _cdiv(a, b):
    return (a + b - 1) // b


def _wrap16(idx):
    """[n] int -> [128, n/16] int16 in the dma_gather wrapped layout."""
    n = len(idx)
    assert n % 16 == 0
    w = idx.reshape(n // 16, 16).T.astype(np.int16)
    return np.ascontiguousarray(np.tile(w, (8, 1)))


def _host_pack(x, edge_index):
    """Bucket edges by (dest block, lo/hi source half); build per-core inputs."""
    row = np.asarray(edge_index[0], dtype=np.int64)
    col = np.asarray(edge_index[1], dtype=np.int64)

    bg = col >> 7                       # global dest block, 0..312
    hi = (row >= SPLIT).astype(np.int64)
    key = bg * 2 + hi
    order = np.argsort(key, kind="stable")
    row_s, col_s, key_s = row[order], col[order], key[order]
    counts = np.bincount(key_s, minlength=NBLK * N_CORES * 2)
    starts = np.concatenate([[0], np.cumsum(counts)])

    n_lo = counts[0::2].reshape(N_CORES, NBLK)   # [core, blk]
    n_hi = counts[1::2].reshape(N_CORES, NBLK)
    C_lo = np.maximum(1, _cdiv(n_lo.max(axis=0), P))   # [blk]
    C_hi = _cdiv(n_hi.max(axis=0), P)                   # [blk], may be 0

    # group geometry (same for all cores)
    meta = {"C_lo": C_lo, "C_hi": C_hi, "groups": []}
    for g in range(NGRP):
        blks = list(range(g * GRP, (g + 1) * GRP))
        K_lo = int(C_lo[blks].sum())
        K_hi = int(C_hi[blks].sum())
        # chunk id -> (block, kind) map in slot order: lo region then hi region
        chunk_blk = []
        for b in blks:
            chunk_blk += [b] * int(C_lo[b])
        for b in blks:
            chunk_blk += [b] * int(C_hi[b])
        meta["groups"].append({"blks": blks, "K_lo": K_lo, "K_hi": K_hi,
                               "K": K_lo + K_hi, "chunk_blk": chunk_blk})

    in_maps = []
    for core in range(N_CORES):
        m = {}
        for g in range(NGRP):
            gi = meta["groups"][g]
            K_lo, K_hi, K = gi["K_lo"], gi["K_hi"], gi["K"]
            idx_lo = np.zeros(K_lo * P, dtype=np.int64)
            idx_hi = np.zeros(K_hi * P, dtype=np.int64)
            colf = np.full(K * P, 999.0, dtype=np.float32)  # slot-ordered
            off_lo = 0
            off_hi = 0
            for kind in (0, 1):
                for b in gi["blks"]:
                    bg_idx = (core * NBLK + b)
                    if bg_idx >= 313:
                        n = 0
                    else:
                        k2 = bg_idx * 2 + kind
                        n = counts[k2]
                    if kind == 0:
                        cap = int(C_lo[b]) * P
                        if n:
                            s0 = starts[k2]
                            idx_lo[off_lo:off_lo + n] = row_s[s0:s0 + n]
                            colf[off_lo:off_lo + n] = (col_s[s0:s0 + n] & 127)
                        off_lo += cap
                    else:
                        cap = int(C_hi[b]) * P
                        if n:
                            s0 = starts[k2]
                            idx_hi[off_hi:off_hi + n] = row_s[s0:s0 + n] - SPLIT
                            colf[K_lo * P + off_hi:K_lo * P + off_hi + n] = (col_s[s0:s0 + n] & 127)
                        off_hi += cap
            # colf covers lo region [0, K_lo*P) then hi region
            m[f"idxlo{g}"] = _wrap16(idx_lo)
            if K_hi:
                m[f"idxhi{g}"] = _wrap16(idx_hi)
            m[f"colf{g}"] = np.ascontiguousarray(
                colf.reshape(K, P).T).astype(np.float32)        # [128, K]
        in_maps.append(m)

    # sanity: each (core,block,kind) run fits its cap
    assert (n_lo <= C_lo[None, :] * P).all() and (n_hi <= C_hi[None, :] * P).all()
    return in_maps, meta


def _build_program(meta, att_b, eps_v):
    nc = bacc.Bacc("TRN2", target_bir_lowering=False, debug=False,
                   enable_asserts=False, num_swdge_queues=4)

    x_d = nc.dram_tensor("x", (N_NODES, D), mybir.dt.float32, kind="ExternalInput")
    xloc_d = nc.dram_tensor("xloc", (NLOC, D), mybir.dt.float32, kind="ExternalInput")
    w1_d = nc.dram_tensor("w1", (P, D), mybir.dt.float32, kind="ExternalInput")
    w2_d = nc.dram_tensor("w2", (P, D), mybir.dt.float32, kind="ExternalInput")
    iota_d = nc.dram_tensor("iota", (P, D), mybir.dt.float32, kind="ExternalInput")
    grp_in = []
    for g, gi in enumerate(meta["groups"]):
        K_lo, K_hi, K = gi["K_lo"], gi["K_hi"], gi["K"]
        dlo = nc.dram_tensor(f"idxlo{g}", (P, K_lo * 8), mybir.dt.int16, kind="ExternalInput")
        dhi = (nc.dram_tensor(f"idxhi{g}", (P, K_hi * 8), mybir.dt.int16, kind="ExternalInput")
               if K_hi else None)
        dcol = nc.dram_tensor(f"colf{g}", (P, K), mybir.dt.float32, kind="ExternalInput")
        grp_in.append((dlo, dhi, dcol))
    out_d = nc.dram_tensor("out", (NLOC, D), mybir.dt.float32, kind="ExternalOutput")

    K_max = max(gi["K"] for gi in meta["groups"])

    with tile.TileContext(nc) as tc, ExitStack() as ctx:
        cpool = ctx.enter_context(tc.tile_pool(name="consts", bufs=1))
        spool = ctx.enter_context(tc.tile_pool(name="small", bufs=2))
        gpool = ctx.enter_context(tc.tile_pool(name="gather", bufs=2))
        mpool = ctx.enter_context(tc.tile_pool(name="mul", bufs=1))
        opool = ctx.enter_context(tc.tile_pool(name="oh", bufs=2))
        apool = ctx.enter_context(tc.tile_pool(name="alpha", bufs=2))
        tbpool = ctx.enter_context(tc.tile_pool(name="tb", bufs=2))
        ppool = ctx.enter_context(tc.tile_pool(name="psum", bufs=4, space="PSUM"))
        dpool = ctx.enter_context(tc.tile_pool(name="dram", bufs=1, space="DRAM"))

        w1_sb = cpool.tile([P, D], mybir.dt.float32)
        w2_sb = cpool.tile([P, D], mybir.dt.float32)
        iota_sb = cpool.tile([P, D], mybir.dt.float32)
        nc.sync.dma_start(out=w1_sb[:], in_=w1_d.ap())
        nc.sync.dma_start(out=w2_sb[:], in_=w2_d.ap())
        nc.sync.dma_start(out=iota_sb[:], in_=iota_d.ap())

        # ---- prologue: t = x_loc @ w2 + b  -> t_dram [NLOC, 1] ----
        t_dram = dpool.tile([NLOC, 1], mybir.dt.float32)
        t_all = cpool.tile([P, NBLK], mybir.dt.float32)
        for b in range(NBLK):
            xb = spool.tile([P, D], mybir.dt.float32, tag="xb_pro")
            nc.sync.dma_start(out=xb[:], in_=xloc_d.ap()[b * P:(b + 1) * P, :])
            mulb = spool.tile([P, D], mybir.dt.float32, tag="mulb_pro")
            nc.vector.tensor_tensor(out=mulb[:], in0=xb[:], in1=w2_sb[:],
                                    op=mybir.AluOpType.mult)
            nc.vector.tensor_reduce(out=t_all[:, b:b + 1], in_=mulb[:],
                                    axis=mybir.AxisListType.X, op=mybir.AluOpType.add)
        nc.vector.tensor_scalar_add(out=t_all[:], in0=t_all[:], scalar1=float(att_b))
        nc.sync.dma_start(out=t_dram[:].rearrange("(b p) one -> p (b one)", p=P),
                          in_=t_all[:])

        # ---- main: per group gather + per block one-hot matmul ----
        for g, gi in enumerate(meta["groups"]):
            K_lo, K_hi, K = gi["K_lo"], gi["K_hi"], gi["K"]
            dlo, dhi, dcol = grp_in[g]

            xi = gpool.tile([P, K_max, D], mybir.dt.float32, tag="xi")
            idxlo_sb = spool.tile([P, K_lo * 8], mybir.dt.int16, tag="idxlo")
            nc.sync.dma_start(out=idxlo_sb[:], in_=dlo.ap())
            colf_sb = spool.tile([P, K_max], mybir.dt.float32, tag="colf")
            nc.sync.dma_start(out=colf_sb[:, :K], in_=dcol.ap())

            nc.gpsimd.dma_gather(
                out_ap=xi[:, 0:K_lo, :], in_ap=x_d.ap()[0:SPLIT, :],
                idxs_ap=idxlo_sb[:], num_idxs=K_lo * P, num_idxs_reg=K_lo * P,
                elem_size=D, single_packet=False, queue_num=(2 * g) % 4,
            )
            if K_hi:
                idxhi_sb = spool.tile([P, K_hi * 8], mybir.dt.int16, tag="idxhi")
                nc.sync.dma_start(out=idxhi_sb[:], in_=dhi.ap())
                nc.gpsimd.dma_gather(
                    out_ap=xi[:, K_lo:K, :], in_ap=x_d.ap()[SPLIT:N_NODES, :],
                    idxs_ap=idxhi_sb[:], num_idxs=K_hi * P, num_idxs_reg=K_hi * P,
                    elem_size=D, single_packet=False, queue_num=(2 * g + 1) % 4,
                )

            # s = rowwise dot(x_i, w1) for the whole group
            mul3 = mpool.tile([P, K_max, D], mybir.dt.float32, tag="mul3")
            nc.vector.tensor_tensor(
                out=mul3[:, :K, :], in0=xi[:, :K, :],
                in1=w1_sb[:].unsqueeze(1).to_broadcast([P, K, D]),
                op=mybir.AluOpType.mult)
            s_sel = spool.tile([P, K_max], mybir.dt.float32, tag="ssel")
            nc.vector.tensor_reduce(out=s_sel[:, :K], in_=mul3[:, :K, :],
                                    axis=mybir.AxisListType.X, op=mybir.AluOpType.add)

            # chunk ids per block: a contiguous lo range and a contiguous hi range
            for b in gi["blks"]:
                chunks = [k for k, cb in enumerate(gi["chunk_blk"]) if cb == b]
                C_b = len(chunks)
                # contiguous runs of chunk ids (lo run + hi run)
                runs = []
                r0 = chunks[0]
                prev = r0
                for k in chunks[1:]:
                    if k != prev + 1:
                        runs.append((r0, prev + 1))
                        r0 = k
                    prev = k
                runs.append((r0, prev + 1))

                t_bc = tbpool.tile([P, P], mybir.dt.float32, tag="tbc")
                nc.sync.dma_start(
                    out=t_bc[:],
                    in_=t_dram[:][b * P:(b + 1) * P, :]
                        .rearrange("p one -> (one p)").unsqueeze(0).to_broadcast([P, P]))

                alphaM = apool.tile([P, C_b, P], mybir.dt.float32, tag="alphaM")
                O = opool.tile([P, C_b, P], mybir.dt.float32, tag="O")
                # batched one-hot 0/1: O[:, j, c] = (iota[c] == col[:, k_j])
                j0 = 0
                for (ka, kb) in runs:
                    n = kb - ka
                    nc.vector.tensor_tensor(
                        out=O[:, j0:j0 + n, :],
                        in0=iota_sb[:].unsqueeze(1).to_broadcast([P, n, P]),
                        in1=colf_sb[:, ka:kb].unsqueeze(2).to_broadcast([P, n, P]),
                        op=mybir.AluOpType.is_equal)
                    j0 += n
                # per-chunk tanh into alphaM slices
                for j, k in enumerate(chunks):
                    nc.scalar.activation(out=alphaM[:, j, :], in_=t_bc[:],
                                         func=mybir.ActivationFunctionType.Tanh,
                                         bias=s_sel[:, k:k + 1], scale=1.0)
                # batched mask multiply
                nc.vector.tensor_tensor(out=O[:], in0=O[:], in1=alphaM[:],
                                        op=mybir.AluOpType.mult)
                psum = ppool.tile([P, D], mybir.dt.float32, space="PSUM", tag="ps")
                for j, k in enumerate(chunks):
                    nc.tensor.matmul(out=psum[:], lhsT=O[:, j, :], rhs=xi[:, k, :],
                                     start=(j == 0), stop=(j == len(chunks) - 1))
                # out = eps * x_loc + (1 - eps) * psum
                xb = spool.tile([P, D], mybir.dt.float32, tag="xb_mix")
                nc.sync.dma_start(out=xb[:], in_=xloc_d.ap()[b * P:(b + 1) * P, :])
                xbe = spool.tile([P, D], mybir.dt.float32, tag="xbe")
                nc.vector.tensor_scalar_mul(out=xbe[:], in0=xb[:], scalar1=float(eps_v))
                osb = spool.tile([P, D], mybir.dt.float32, tag="osb")
                nc.vector.scalar_tensor_tensor(
                    out=osb[:], in0=psum[:], scalar=float(1.0 - eps_v), in1=xbe[:],
                    op0=mybir.AluOpType.mult, op1=mybir.AluOpType.add)
                nc.sync.dma_start(out=out_d.ap()[b * P:(b + 1) * P, :], in_=osb[:])

    nc.compile()
    return nc


def _run(inputs, trace=False, trace_kwargs=None):
    x = np.ascontiguousarray(np.asarray(inputs["x"], dtype=np.float32))
    edge_index = np.asarray(inputs["edge_index"])
    att_w = np.asarray(inputs["att_w"], dtype=np.float32)
    att_b = float(np.asarray(inputs["att_b"], dtype=np.float32)[0])
    eps_v = float(np.asarray(inputs["eps"], dtype=np.float32)[0])

    in_maps, meta = _host_pack(x, edge_index)

    w1 = np.ascontiguousarray(np.tile(att_w[:, :D], (P, 1)))
    w2 = np.ascontiguousarray(np.tile(att_w[:, D:], (P, 1)))
    iota = np.ascontiguousarray(np.tile(np.arange(P, dtype=np.float32)[None, :], (P, 1)))
    x_pad = np.zeros((NPAD, D), dtype=np.float32)
    x_pad[:N_NODES] = x

    for core in range(N_CORES):
        m = in_maps[core]
        m["x"] = x
        m["xloc"] = np.ascontiguousarray(x_pad[core * NLOC:(core + 1) * NLOC])
        m["w1"] = w1
        m["w2"] = w2
        m["iota"] = iota

    nc = _build_program(meta, att_b, eps_v)
    res = bu.run_bass_kernel_spmd(nc, in_maps, list(range(N_CORES)),
                                  trace=trace, **(trace_kwargs or {}))
    out = np.concatenate([res.results[c]["out"] for c in range(N_CORES)], axis=0)
    return np.ascontiguousarray(out[:N_NODES]), res


def kernel(**inputs) -> np.ndarray:
    out, _ = _run(inputs, trace=False)
    return out


# revision 7
# speedup vs baseline: 1.5915x; 1.2837x over previous
"""FAGCN layer (gnn_message_passing) as a Bass/Tile kernel on 8 trn2 NeuronCores.

Strategy: destination-sharded edge parallelism (no collectives).
  - Node dest range is split into 8 contiguous slices of 5120 nodes (40000
    padded to 40960); core k owns all edges whose dest (col) lands in its slice.
  - x is replicated; each core gathers the source rows x[row] it needs with
    dma_gather (int16 indices => per-block lo/hi split at 32768).
  - Edges are bucketed by dest block of 128 nodes; a chunk of 128 edges is
    scattered into the block via a one-hot matmul accumulated in PSUM:
        psum[c, f] += sum_e onehot[e, c] * x_i[e, f],  onehot[e, col_e] = alpha_e
  - alpha_e = tanh(s_e + t_col + b) with s = x_i . w1 computed by a fused
    mul+reduce on DVE, and t = x_loc . w2 + b precomputed per core; the
    per-edge t selection is fused into the ACT engine:
        alphaM[e, c] = tanh(t_bcast[c] + s_e)  (bias = per-partition s)
    and the one-hot build picks column col_e:
        O[e, c] = (iota[c] == col_e) * alphaM[e, c]
  - out = eps * x + (1 - eps) * psum, written per block; host concatenates the
    8 core slices.
"""
import numpy as np
from contextlib import ExitStack

import concourse.bass as bass
import concourse.bacc as bacc
import concourse.mybir as mybir
import concourse.tile as tile
import concourse.bass_utils as bu

P = 128
D = 128
N_NODES = 40000
N_EDGES = 640000
N_CORES = 8
NPAD = 40960
NLOC = NPAD // N_CORES          # 5120 dest nodes per core
NBLK = NLOC // P                # 40 dest blocks per core
SPLIT = 32768                   # int16 gather index limit
GRP = 4                         # dest blocks per gather group
NGRP = NBLK // GRP


def _cdiv(a, b):
    return (a + b - 1) // b


def _wrap16(idx):
    """[n] int -> [128, n/16] int16 in the dma_gather wrapped layout."""
    n = len(idx)
    assert n % 16 == 0
    w = idx.reshape(n // 16, 16).T.astype(np.int16)
    return np.ascontiguousarray(np.tile(w, (8, 1)))


def _host_pack(x, edge_index):
    """Bucket edges by (dest block, lo/hi source half); build per-core inputs."""
    row = np.asarray(edge_index[0], dtype=np.int64)
    col = np.asarray(edge_index[1], dtype=np.int64)

    bg = col >> 7                       # global dest block, 0..312
    hi = (row >= SPLIT).astype(np.int64)
    key = bg * 2 + hi
    order = np.argsort(key, kind="stable")
    row_s, col_s, key_s = row[order], col[order], key[order]
    counts = np.bincount(key_s, minlength=NBLK * N_CORES * 2)
    starts = np.concatenate([[0], np.cumsum(counts)])

    n_lo = counts[0::2].reshape(N_CORES, NBLK)   # [core, blk]
    n_hi = counts[1::2].reshape(N_CORES, NBLK)
    C_lo = np.maximum(1, _cdiv(n_lo.max(axis=0), P))   # [blk]
    C_hi = _cdiv(n_hi.max(axis=0), P)                   # [blk], may be 0

    # group geometry (same for all cores)
    meta = {"C_lo": C_lo, "C_hi": C_hi, "groups": []}
    for g in range(NGRP):
        blks = list(range(g * GRP, (g + 1) * GRP))
        K_lo = int(C_lo[blks].sum())
        K_hi = int(C_hi[blks].sum())
        # chunk id -> (block, kind) map in slot order: lo region then hi region
        chunk_blk = []
        for b in blks:
            chunk_blk += [b] * int(C_lo[b])
        for b in blks:
            chunk_blk += [b] * int(C_hi[b])
        meta["groups"].append({"blks": blks, "K_lo": K_lo, "K_hi": K_hi,
                               "K": K_lo + K_hi, "chunk_blk": chunk_blk})

    in_maps = []
    for core in range(N_CORES):
        m = {}
        for g in range(NGRP):
            gi = meta["groups"][g]
            K_lo, K_hi, K = gi["K_lo"], gi["K_hi"], gi["K"]
            idx_lo = np.zeros(K_lo * P, dtype=np.int64)
            idx_hi = np.zeros(K_hi * P, dtype=np.int64)
            colf = np.full(K * P, 999.0, dtype=np.float32)  # slot-ordered
            off_lo = 0
            off_hi = 0
            for kind in (0, 1):
                for b in gi["blks"]:
                    bg_idx = (core * NBLK + b)
                    if bg_idx >= 313:
                        n = 0
                    else:
                        k2 = bg_idx * 2 + kind
                        n = counts[k2]
                    if kind == 0:
                        cap = int(C_lo[b]) * P
                        if n:
                            s0 = starts[k2]
                            idx_lo[off_lo:off_lo + n] = row_s[s0:s0 + n]
                            colf[off_lo:off_lo + n] = (col_s[s0:s0 + n] & 127)
                        off_lo += cap
                    else:
                        cap = int(C_hi[b]) * P
                        if n:
                            s0 = starts[k2]
                            idx_hi[off_hi:off_hi + n] = row_s[s0:s0 + n] - SPLIT
                            colf[K_lo * P + off_hi:K_lo * P + off_hi + n] = (col_s[s0:s0 + n] & 127)
                        off_hi += cap
            # colf covers lo region [0, K_lo*P) then hi region
            m[f"idxlo{g}"] = _wrap16(idx_lo)
            if K_hi:
                m[f"idxhi{g}"] = _wrap16(idx_hi)
            m[f"colf{g}"] = np.ascontiguousarray(
                colf.reshape(K, P).T).astype(np.float32)        # [128, K]
        in_maps.append(m)

    # sanity: each (core,block,kind) run fits its cap
    assert (n_lo <= C_lo[None, :] * P).all() and (n_hi <= C_hi[None, :] * P).all()
    return in_maps, meta


def _build_program(meta, att_b, eps_v):
    nc = bacc.Bacc("TRN2", target_bir_lowering=False, debug=False,
                   enable_asserts=False, num_swdge_queues=4)

    x_d = nc.dram_tensor("x", (N_NODES, D), mybir.dt.float32, kind="ExternalInput")
    xloc_d = nc.dram_tensor("xloc", (NLOC, D), mybir.dt.float32, kind="ExternalInput")
    w1_d = nc.dram_tensor("w1", (P, D), mybir.dt.float32, kind="ExternalInput")
    w2_d = nc.dram_tensor("w2", (P, D), mybir.dt.float32, kind="ExternalInput")
    iota_d = nc.dram_tensor("iota", (P, D), mybir.dt.float32, kind="ExternalInput")
    grp_in = []
    for g, gi in enumerate(meta["groups"]):
        K_lo, K_hi, K = gi["K_lo"], gi["K_hi"], gi["K"]
        dlo = nc.dram_tensor(f"idxlo{g}", (P, K_lo * 8), mybir.dt.int16, kind="ExternalInput")
        dhi = (nc.dram_tensor(f"idxhi{g}", (P, K_hi * 8), mybir.dt.int16, kind="ExternalInput")
               if K_hi else None)
        dcol = nc.dram_tensor(f"colf{g}", (P, K), mybir.dt.float32, kind="ExternalInput")
        grp_in.append((dlo, dhi, dcol))
    out_d = nc.dram_tensor("out", (NLOC, D), mybir.dt.float32, kind="ExternalOutput")

    K_max = max(gi["K"] for gi in meta["groups"])

    with tile.TileContext(nc) as tc, ExitStack() as ctx:
        cpool = ctx.enter_context(tc.tile_pool(name="consts", bufs=1))
        spool = ctx.enter_context(tc.tile_pool(name="small", bufs=2))
        gpool = ctx.enter_context(tc.tile_pool(name="gather", bufs=2))
        mpool = ctx.enter_context(tc.tile_pool(name="mul", bufs=1))
        opool = ctx.enter_context(tc.tile_pool(name="oh", bufs=2))
        apool = ctx.enter_context(tc.tile_pool(name="alpha", bufs=2))
        tbpool = ctx.enter_context(tc.tile_pool(name="tb", bufs=2))
        ppool = ctx.enter_context(tc.tile_pool(name="psum", bufs=4, space="PSUM"))
        dpool = ctx.enter_context(tc.tile_pool(name="dram", bufs=1, space="DRAM"))

        w1_sb = cpool.tile([P, D], mybir.dt.float32)
        w2_sb = cpool.tile([P, D], mybir.dt.float32)
        iota_sb = cpool.tile([P, D], mybir.dt.float32)
        nc.sync.dma_start(out=w1_sb[:], in_=w1_d.ap())
        nc.sync.dma_start(out=w2_sb[:], in_=w2_d.ap())
        nc.sync.dma_start(out=iota_sb[:], in_=iota_d.ap())

        # ---- prologue: t = x_loc @ w2 + b  -> t_dram [NLOC, 1] ----
        t_dram = dpool.tile([NLOC, 1], mybir.dt.float32)
        t_all = cpool.tile([P, NBLK], mybir.dt.float32)
        for b in range(NBLK):
            xb = spool.tile([P, D], mybir.dt.float32, tag="xb_pro")
            nc.sync.dma_start(out=xb[:], in_=xloc_d.ap()[b * P:(b + 1) * P, :])
            mulb = spool.tile([P, D], mybir.dt.float32, tag="mulb_pro")
            nc.vector.tensor_tensor(out=mulb[:], in0=xb[:], in1=w2_sb[:],
                                    op=mybir.AluOpType.mult)
            nc.vector.tensor_reduce(out=t_all[:, b:b + 1], in_=mulb[:],
                                    axis=mybir.AxisListType.X, op=mybir.AluOpType.add)
        nc.vector.tensor_scalar_add(out=t_all[:], in0=t_all[:], scalar1=float(att_b))
        nc.sync.dma_start(out=t_dram[:].rearrange("(b p) one -> p (b one)", p=P),
                          in_=t_all[:])

        # ---- main: per group gather + per block one-hot matmul ----
        for g, gi in enumerate(meta["groups"]):
            K_lo, K_hi, K = gi["K_lo"], gi["K_hi"], gi["K"]
            dlo, dhi, dcol = grp_in[g]

            xi = gpool.tile([P, K_max, D], mybir.dt.float32, tag="xi")
            idxlo_sb = spool.tile([P, K_lo * 8], mybir.dt.int16, tag="idxlo")
            nc.sync.dma_start(out=idxlo_sb[:], in_=dlo.ap())
            colf_sb = spool.tile([P, K_max], mybir.dt.float32, tag="colf")
            nc.sync.dma_start(out=colf_sb[:, :K], in_=dcol.ap())

            # split the big lo gather in two and spread all gathers round-robin
            # across the 4 SWDGE queues for better Q7 overlap
            qn = 3 * g
            half = (K_lo // 2) & ~0  # chunk-aligned halves
            for (c0, c1) in ((0, half), (half, K_lo)):
                if c1 > c0:
                    nc.gpsimd.dma_gather(
                        out_ap=xi[:, c0:c1, :], in_ap=x_d.ap()[0:SPLIT, :],
                        idxs_ap=idxlo_sb[:, c0 * 8:c1 * 8],
                        num_idxs=(c1 - c0) * P, num_idxs_reg=(c1 - c0) * P,
                        elem_size=D, single_packet=False, queue_num=qn % 4,
                    )
                    qn += 1
            if K_hi:
                idxhi_sb = spool.tile([P, K_hi * 8], mybir.dt.int16, tag="idxhi")
                nc.sync.dma_start(out=idxhi_sb[:], in_=dhi.ap())
                nc.gpsimd.dma_gather(
                    out_ap=xi[:, K_lo:K, :], in_ap=x_d.ap()[SPLIT:N_NODES, :],
                    idxs_ap=idxhi_sb[:], num_idxs=K_hi * P, num_idxs_reg=K_hi * P,
                    elem_size=D, single_packet=False, queue_num=qn % 4,
                )

            # s = rowwise dot(x_i, w1) for the whole group
            mul3 = mpool.tile([P, K_max, D], mybir.dt.float32, tag="mul3")
            nc.vector.tensor_tensor(
                out=mul3[:, :K, :], in0=xi[:, :K, :],
                in1=w1_sb[:].unsqueeze(1).to_broadcast([P, K, D]),
                op=mybir.AluOpType.mult)
            s_sel = spool.tile([P, K_max], mybir.dt.float32, tag="ssel")
            nc.vector.tensor_reduce(out=s_sel[:, :K], in_=mul3[:, :K, :],
                                    axis=mybir.AxisListType.X, op=mybir.AluOpType.add)

            # chunk ids per block: a contiguous lo range and a contiguous hi range
            for b in gi["blks"]:
                chunks = [k for k, cb in enumerate(gi["chunk_blk"]) if cb == b]
                C_b = len(chunks)
                # contiguous runs of chunk ids (lo run + hi run)
                runs = []
                r0 = chunks[0]
                prev = r0
                for k in chunks[1:]:
                    if k != prev + 1:
                        runs.append((r0, prev + 1))
                        r0 = k
                    prev = k
                runs.append((r0, prev + 1))

                t_bc = tbpool.tile([P, P], mybir.dt.float32, tag="tbc")
                nc.sync.dma_start(
                    out=t_bc[:],
                    in_=t_dram[:][b * P:(b + 1) * P, :]
                        .rearrange("p one -> (one p)").unsqueeze(0).to_broadcast([P, P]))

                alphaM = apool.tile([P, C_b, P], mybir.dt.float32, tag="alphaM")
                O = opool.tile([P, C_b, P], mybir.dt.float32, tag="O")
                # batched one-hot 0/1: O[:, j, c] = (iota[c] == col[:, k_j])
                j0 = 0
                for (ka, kb) in runs:
                    n = kb - ka
                    nc.vector.tensor_tensor(
                        out=O[:, j0:j0 + n, :],
                        in0=iota_sb[:].unsqueeze(1).to_broadcast([P, n, P]),
                        in1=colf_sb[:, ka:kb].unsqueeze(2).to_broadcast([P, n, P]),
                        op=mybir.AluOpType.is_equal)
                    j0 += n
                # per-chunk tanh into alphaM slices
                for j, k in enumerate(chunks):
                    nc.scalar.activation(out=alphaM[:, j, :], in_=t_bc[:],
                                         func=mybir.ActivationFunctionType.Tanh,
                                         bias=s_sel[:, k:k + 1], scale=1.0)
                # batched mask multiply
                nc.vector.tensor_tensor(out=O[:], in0=O[:], in1=alphaM[:],
                                        op=mybir.AluOpType.mult)
                psum = ppool.tile([P, D], mybir.dt.float32, space="PSUM", tag="ps")
                for j, k in enumerate(chunks):
                    nc.tensor.matmul(out=psum[:], lhsT=O[:, j, :], rhs=xi[:, k, :],
                                     start=(j == 0), stop=(j == len(chunks) - 1))
                # out = eps * x_loc + (1 - eps) * psum
                xb = spool.tile([P, D], mybir.dt.float32, tag="xb_mix")
                nc.sync.dma_start(out=xb[:], in_=xloc_d.ap()[b * P:(b + 1) * P, :])
                xbe = spool.tile([P, D], mybir.dt.float32, tag="xbe")
                nc.vector.tensor_scalar_mul(out=xbe[:], in0=xb[:], scalar1=float(eps_v))
                osb = spool.tile([P, D], mybir.dt.float32, tag="osb")
                nc.vector.scalar_tensor_tensor(
                    out=osb[:], in0=psum[:], scalar=float(1.0 - eps_v), in1=xbe[:],
                    op0=mybir.AluOpType.mult, op1=mybir.AluOpType.add)
                nc.sync.dma_start(out=out_d.ap()[b * P:(b + 1) * P, :], in_=osb[:])

    nc.compile()
    return nc


def _run(inputs, trace=False, trace_kwargs=None):
    x = np.ascontiguousarray(np.asarray(inputs["x"], dtype=np.float32))
    edge_index = np.asarray(inputs["edge_index"])
    att_w = np.asarray(inputs["att_w"], dtype=np.float32)
    att_b = float(np.asarray(inputs["att_b"], dtype=np.float32)[0])
    eps_v = float(np.asarray(inputs["eps"], dtype=np.float32)[0])

    in_maps, meta = _host_pack(x, edge_index)

    w1 = np.ascontiguousarray(np.tile(att_w[:, :D], (P, 1)))
    w2 = np.ascontiguousarray(np.tile(att_w[:, D:], (P, 1)))
    iota = np.ascontiguousarray(np.tile(np.arange(P, dtype=np.float32)[None, :], (P, 1)))
    x_pad = np.zeros((NPAD, D), dtype=np.float32)
    x_pad[:N_NODES] = x

    for core in range(N_CORES):
        m = in_maps[core]
        m["x"] = x
        m["xloc"] = np.ascontiguousarray(x_pad[core * NLOC:(core + 1) * NLOC])
        m["w1"] = w1
        m["w2"] = w2
        m["iota"] = iota

    nc = _build_program(meta, att_b, eps_v)
    res = bu.run_bass_kernel_spmd(nc, in_maps, list(range(N_CORES)),
                                  trace=trace, **(trace_kwargs or {}))
    out = np.concatenate([res.results[c]["out"] for c in range(N_CORES)], axis=0)
    return np.ascontiguousarray(out[:N_NODES]), res


def kernel(**inputs) -> np.ndarray:
    out, _ = _run(inputs, trace=False)
    return out
